# revision 1
# baseline (speedup 1.0000x reference)
"""Trainium2 Bass kernel for nn_LinearLoopLayer: out = x @ weight.T + bias.

x: (2048, 4096) f32, weight: (4096, 4096) f32, bias: (4096,) f32.
Sharding: 2 batch-halves x 4 out-feature-quarters across 8 NeuronCores.
Each core computes outT_shard[j, b] = sum_i wT[i, j] * xT[i, b] + bias[j]
with host-pre-transposed xT/wT so the contraction dim i is the SBUF
partition dim (no on-device transposes).

Matmuls run in float32r (full-rate PE mode for 4-byte floats, ~1e-4 rel
err vs ~3e-7 for plain float32 at 1/4 rate; flip with LINEAR_MM_DT=f32).

DMA ring use: xt + out on the sync (SP) HWDGE ring, wt on the scalar
(ACT) ring, so pass-1 weight tiles aren't queued behind the 16MB xt
stream (HWDGE is FIFO per issuing engine). xt loads are interleaved
into the pass-1 compute loop to keep ring order = consumption order.
"""

import os
import sys

import numpy as np

sys.path.insert(0, "/opt/trn_rl_repo")

import concourse.mybir as mybir
from concourse import bacc, tile
from concourse.bass_utils import run_bass_kernel_spmd

P = 128
B, K, J = 2048, 4096, 4096
NCORES = 8
B_SPLIT, J_SPLIT = 2, 4
BL, JL = B // B_SPLIT, J // J_SPLIT  # per-core local batch / out-features
KT = K // P  # contraction tiles
NB = BL // 512  # moving-dim (batch) blocks per core
JS = JL // 512  # j-super blocks (512 features) per core
JSUB = 512 // P  # 128-feature psum row-blocks per j-super

_DT_BY_NAME = {
    "f32": mybir.dt.float32,
    "f32r": mybir.dt.float32r,
    "bf16": mybir.dt.bfloat16,
}
_MM_DT_NAME = os.environ.get("LINEAR_MM_DT", "f32r")


def _build(mm_dt):
    """One SPMD program; per-core differences come only via input shards."""
    nc = bacc.Bacc(None, target_bir_lowering=False)
    xt = nc.declare_dram_parameter("xt", [K, BL], mm_dt, isOutput=False)
    wt = nc.declare_dram_parameter("wt", [K, JL], mm_dt, isOutput=False)
    biasT = nc.declare_dram_parameter(
        "biasT", [P, JL // P], mybir.dt.float32, isOutput=False
    )
    out = nc.declare_dram_parameter("out", [JL, BL], mybir.dt.float32, isOutput=True)

    f32 = mybir.dt.float32
    with tile.TileContext(nc) as tc:
        with (
            tc.tile_pool(name="xtp", bufs=KT) as xt_pool,
            tc.tile_pool(name="wtp", bufs=6) as wt_pool,
            tc.tile_pool(name="outp", bufs=4) as out_pool,
            tc.tile_pool(name="biasp", bufs=1) as bias_pool,
            tc.tile_pool(name="psum", bufs=8, space="PSUM") as psum_pool,
        ):
            bias_sb = bias_pool.tile([P, JL // P], f32)
            nc.sync.dma_start(bias_sb[:], biasT[:, :])

            xt_tiles = [None] * KT
            for js in range(JS):
                ps = [
                    [psum_pool.tile([P, 512], f32, name="ps") for bb in range(NB)]
                    for jsub in range(JSUB)
                ]
                for i in range(KT):
                    if js == 0:
                        # xt shard stays resident; loaded in consumption order
                        t = xt_pool.tile([P, BL], mm_dt, name="xt")
                        nc.sync.dma_start(t[:], xt[i * P : (i + 1) * P, :])
                        xt_tiles[i] = t
                    wt_t = wt_pool.tile([P, 512], mm_dt, name="wt")
                    nc.scalar.dma_start(
                        wt_t[:], wt[i * P : (i + 1) * P, js * 512 : (js + 1) * 512]
                    )
                    for jsub in range(JSUB):
                        for bb in range(NB):
                            nc.tensor.matmul(
                                ps[jsub][bb][:],
                                wt_t[:, jsub * P : (jsub + 1) * P],
                                xt_tiles[i][:, bb * 512 : (bb + 1) * 512],
                                start=(i == 0),
                                stop=(i == KT - 1),
                            )
                for jsub in range(JSUB):
                    jb = js * JSUB + jsub
                    for bb in range(NB):
                        o = out_pool.tile([P, 512], f32, name="o")
                        nc.vector.tensor_scalar_add(
                            o[:], ps[jsub][bb][:], bias_sb[:, jb : jb + 1]
                        )
                        nc.sync.dma_start(
                            out[jb * P : (jb + 1) * P, bb * 512 : (bb + 1) * 512], o[:]
                        )
    nc.finalize()
    return nc


_NC_CACHE = {}


def _get_nc(mm_dt_name):
    if mm_dt_name not in _NC_CACHE:
        _NC_CACHE[mm_dt_name] = _build(_DT_BY_NAME[mm_dt_name])
    return _NC_CACHE[mm_dt_name]


def _make_in_maps(x, weight, bias):
    x = np.asarray(x, dtype=np.float32)
    if x.ndim == 4:
        x = x.reshape(x.shape[0], -1)
    weight = np.asarray(weight, dtype=np.float32)
    bias = np.asarray(bias, dtype=np.float32)
    in_maps = []
    for c in range(NCORES):
        bh, jq = divmod(c, J_SPLIT)
        bq = bias[jq * JL : (jq + 1) * JL]
        in_maps.append(
            {
                "xt": np.ascontiguousarray(x[bh * BL : (bh + 1) * BL].T),
                "wt": np.ascontiguousarray(weight[jq * JL : (jq + 1) * JL].T),
                "biasT": np.ascontiguousarray(bq.reshape(JL // P, P).T),
            }
        )
    return in_maps


def _assemble(results):
    out = np.empty((B, J), dtype=np.float32)
    for c in range(NCORES):
        bh, jq = divmod(c, J_SPLIT)
        out[bh * BL : (bh + 1) * BL, jq * JL : (jq + 1) * JL] = results[c]["out"].T
    return out


def run(x, weight, bias, mm_dt_name=None, trace=False, **kwargs):
    nc = _get_nc(mm_dt_name or _MM_DT_NAME)
    in_maps = _make_in_maps(x, weight, bias)
    res = run_bass_kernel_spmd(
        nc, in_maps, core_ids=list(range(NCORES)), trace=trace, **kwargs
    )
    return _assemble(res.results), res


def kernel(x, weight, bias):
    out, _ = run(x, weight, bias)
    return out



# revision 2
# speedup vs baseline: 1.2216x; 1.2216x over previous
"""Trainium2 Bass kernel for nn_LinearLoopLayer: out = x @ weight.T + bias.

x: (2048, 4096) f32, weight: (4096, 4096) f32, bias: (4096,) f32.
Sharding: 2 batch-halves x 4 out-feature-quarters across 8 NeuronCores.
Each core computes outT_shard[j, b] = sum_i wT[i, j] * xT[i, b] + bias[j].

The baseline f32r version was DMA-bound (37.8 MB/core at ~247 GB/s vs a
~110 us PE floor): DMA active 99%, PE 77%, with ~18 us of HAM cold-clock
from PE idle gaps. This version:
  - converts x/w to bf16 on the host (input DMA halves to 16.8 MB/core;
    quantization rel-err ~2.2e-3 vs the 2e-2 gate; PE rate unchanged)
  - pre-transposes shards to partition-major layout so every DMA is
    long contiguous runs per partition, issued as 256KB..2MB chunks
  - keeps x + w fully SBUF-resident (136 KB/partition)
  - streams in consumption order: wtA (first 512 out-features) on the
    ACT ring; xt then wtB on the SP ring, so pass-A weights never
    queue behind pass-B bytes
  - accumulates in 3 psum passes (8/4/4 banks) over all of K, so the
    exposed store tail is only ~1 MB
"""

import sys

import numpy as np

sys.path.insert(0, "/opt/trn_rl_repo")

import concourse.mybir as mybir
from concourse import bacc, tile
from concourse.bass_utils import run_bass_kernel_spmd

P = 128
B, K, J = 2048, 4096, 4096
NCORES = 8
B_SPLIT, J_SPLIT = 2, 4
BL, JL = B // B_SPLIT, J // J_SPLIT  # per-core local batch / out-features
KT = K // P  # contraction tiles (32)
JB = JL // P  # 128-feature j-blocks per core (8)
NB = BL // 512  # 512-col batch blocks per core (2)
JH = JL // 2  # out-feature half (512) - wtA/wtB split

# psum passes: j-block groups of 4/2/2 (x NB banks each = 8/4/4 banks)
PASSES = [(0, 1, 2, 3), (4, 5), (6, 7)]
# chunk sizes in i-tiles: fine-grained early so compute starts ~2us in
XT_CHUNKS = [1, 1, 1, 1, 4, 8, 8, 8]
WA_CHUNKS = [1, 1, 2, 4, 8, 8, 8]
WB_CHUNKS = [8, 8, 8, 8]

_NP_BF16 = mybir.dt.np(mybir.dt.bfloat16)


def _chunk_offsets(sizes):
    off, out = 0, []
    for n in sizes:
        out.append((off, n))
        off += n
    return out


def _build():
    nc = bacc.Bacc(None, target_bir_lowering=False)
    bf16 = mybir.dt.bfloat16
    f32 = mybir.dt.float32
    xt = nc.declare_dram_parameter("xt", [P, KT * BL], bf16, isOutput=False)
    wta = nc.declare_dram_parameter("wta", [P, KT * JH], bf16, isOutput=False)
    wtb = nc.declare_dram_parameter("wtb", [P, KT * JH], bf16, isOutput=False)
    biasT = nc.declare_dram_parameter("biasT", [P, JB], f32, isOutput=False)
    out = nc.declare_dram_parameter("out", [JL, BL], f32, isOutput=True)

    with tile.TileContext(nc) as tc:
        with (
            tc.tile_pool(name="xp", bufs=1) as xp,
            tc.tile_pool(name="wap", bufs=1) as wap,
            tc.tile_pool(name="wbp", bufs=1) as wbp,
            tc.tile_pool(name="biasp", bufs=1) as biasp,
            tc.tile_pool(name="outp", bufs=4) as outp,
            tc.tile_pool(name="psum", bufs=8, space="PSUM") as psum_pool,
        ):
            xt_sb = xp.tile([P, KT * BL], bf16)
            wta_sb = wap.tile([P, KT * JH], bf16)
            wtb_sb = wbp.tile([P, KT * JH], bf16)
            bias_sb = biasp.tile([P, JB], f32)

            nc.scalar.dma_start(bias_sb[:], biasT[:, :])
            # pass-A weights on the ACT ring, in consumption order
            for off, n in _chunk_offsets(WA_CHUNKS):
                nc.scalar.dma_start(
                    wta_sb[:, off * JH : (off + n) * JH],
                    wta[:, off * JH : (off + n) * JH],
                )
            # x then pass-B/C weights on the SP ring (FIFO: xt bytes first)
            for off, n in _chunk_offsets(XT_CHUNKS):
                nc.sync.dma_start(
                    xt_sb[:, off * BL : (off + n) * BL],
                    xt[:, off * BL : (off + n) * BL],
                )
            for off, n in _chunk_offsets(WB_CHUNKS):
                nc.sync.dma_start(
                    wtb_sb[:, off * JH : (off + n) * JH],
                    wtb[:, off * JH : (off + n) * JH],
                )

            for pass_jbs in PASSES:
                ps = {
                    (jb, bb): psum_pool.tile([P, 512], f32, name="ps")
                    for jb in pass_jbs
                    for bb in range(NB)
                }
                for it in range(KT):
                    for jb in pass_jbs:
                        wsrc = wta_sb if jb < 4 else wtb_sb
                        jo = it * JH + (jb % 4) * P
                        st = wsrc[:, jo : jo + P]
                        for bb in range(NB):
                            nc.tensor.matmul(
                                ps[(jb, bb)][:],
                                st,
                                xt_sb[:, it * BL + bb * 512 : it * BL + (bb + 1) * 512],
                                start=(it == 0),
                                stop=(it == KT - 1),
                            )
                for jb in pass_jbs:
                    for bb in range(NB):
                        o = outp.tile([P, 512], f32, name="o")
                        nc.vector.tensor_scalar_add(
                            o[:], ps[(jb, bb)][:], bias_sb[:, jb : jb + 1]
                        )
                        nc.sync.dma_start(
                            out[jb * P : (jb + 1) * P, bb * 512 : (bb + 1) * 512], o[:]
                        )
    nc.finalize()
    return nc


_NC_CACHE = {}


def _get_nc():
    if "bf16" not in _NC_CACHE:
        _NC_CACHE["bf16"] = _build()
    return _NC_CACHE["bf16"]


def _part_major(a2d, cols):
    """[K, cols] f32 -> [P, KT*cols] bf16, i-tile-then-col per partition."""
    return np.ascontiguousarray(
        a2d.reshape(KT, P, cols).transpose(1, 0, 2).reshape(P, KT * cols)
    ).astype(_NP_BF16)


def _make_in_maps(x, weight, bias):
    x = np.asarray(x, dtype=np.float32)
    if x.ndim == 4:
        x = x.reshape(x.shape[0], -1)
    weight = np.asarray(weight, dtype=np.float32)
    bias = np.asarray(bias, dtype=np.float32)
    in_maps = []
    for c in range(NCORES):
        bh, jq = divmod(c, J_SPLIT)
        xT = x[bh * BL : (bh + 1) * BL].T  # [K, BL]
        wT = weight[jq * JL : (jq + 1) * JL].T  # [K, JL]
        bq = bias[jq * JL : (jq + 1) * JL]
        in_maps.append(
            {
                "xt": _part_major(xT, BL),
                "wta": _part_major(wT[:, :JH], JH),
                "wtb": _part_major(wT[:, JH:], JH),
                "biasT": np.ascontiguousarray(bq.reshape(JB, P).T),
            }
        )
    return in_maps


def _assemble(results):
    out = np.empty((B, J), dtype=np.float32)
    for c in range(NCORES):
        bh, jq = divmod(c, J_SPLIT)
        out[bh * BL : (bh + 1) * BL, jq * JL : (jq + 1) * JL] = results[c]["out"].T
    return out


def run(x, weight, bias, mm_dt_name=None, trace=False, **kwargs):
    nc = _get_nc()
    in_maps = _make_in_maps(x, weight, bias)
    res = run_bass_kernel_spmd(
        nc, in_maps, core_ids=list(range(NCORES)), trace=trace, **kwargs
    )
    return _assemble(res.results), res


def kernel(x, weight, bias):
    out, _ = run(x, weight, bias)
    return out
